# revision 4
# baseline (speedup 1.0000x reference)
"""Trainium2 Bass kernel for single-head attention with query-axis softmax.

Problem (B=4, S=2048, D=1024):
    q = seq1 @ Wq^T ; k = seq2 @ Wk^T ; v = seq2 @ Wv^T
    score = q @ k^T / sqrt(D)
    mask_score = where(attn_mask, 1e-9, score)
    p = softmax(mask_score, axis=1)          # softmax over the QUERY axis
    out = p @ v

Math: softmax over q means p[q,k] = exp(s[q,k]) / Z[k] with
Z[k] = sum_q exp(s[q,k]) (no max-subtraction needed: |s| <= ~3, and
exp(1e-9) == 1.0f == exp(0.0) in fp32, so masked entries are exactly
reproduced by zeroing the score). Then out = E @ (v / Z), E = exp(s_masked).

Key algebraic fold: score = seq1 @ (Wq^T Wk) @ seq2^T, so the host
precomputes M = Wq^T @ Wk once and the kernel computes t = seq1 @ M and
score = t @ seq2^T — the K projection is eliminated entirely (the score
contraction runs over the already-resident seq2^T chunks).

Sharding: 8 cores = 4 batches x 2 key-halves. Each core computes the
partial out for its key half; the host sums the two halves per batch.
Scores are built TRANSPOSED (k on partitions, q on the free axis) so the
query-axis softmax is a free-axis reduction fused into the Exp activation
(accum_out), and the 1/sqrt(D) scale rides the activation's scale input.

The t projection is sharded across each core pair by hidden half — the
asymmetry lives in the DATA (each core's wqt input holds only its 512 M
columns), keeping the program SPMD-identical. Partial t^T halves are
exchanged with two pipelined pairwise HBM AllGathers hidden behind the
V-projection phase.

Precision: projections and the out matmul run fp16 (TensorE 1 row/cycle,
fp32 PSUM accumulation). The score matmul runs fp8 e4m3 in DoubleRow
perf mode (2 contraction chunks per instruction, double pump): t^T is
cast to fp8 on-chip (also halving the AllGather wire), seq2^T arrives
from the host in both fp16 (V projection) and fp8 (score stationary).
Measured end-to-end rel err ~1.5e-2 (fp16-only fallback: ~4e-4).
"""

import numpy as np
import ml_dtypes

import concourse.bass as bass
import concourse.tile as tile
from concourse import bacc, mybir
from concourse import bass_utils

B, S, D = 4, 2048, 1024
KSPLIT = 2
KH = S // KSPLIT            # 1024 keys per core
HL = D // 2                 # 512 M-columns of t projected locally
P = 128                     # partitions
DC = D // P                 # 8 contraction chunks (d)
HC = D // P                 # 8 hidden (d') chunks
HCL = HL // P               # 4 local hidden chunks for t^T
KC = KH // P                # 8 key chunks
QN = S // 512               # 4 q tiles of 512
KN = KH // 512              # 2 k tiles of 512
HN = D // 512               # 2 h tiles of 512

F16 = mybir.dt.float16
F32 = mybir.dt.float32
F8 = mybir.dt.float8e4
U8 = mybir.dt.uint8

SCORES_FP8 = False

_NC = {}


def _emit(nc, fp8):
    import contextlib

    qdt = F8 if fp8 else F16

    s1t = nc.dram_tensor("s1t", [D, S], F16, kind="ExternalInput").ap()
    s2t = nc.dram_tensor("s2t", [D, KH], F16, kind="ExternalInput").ap()
    wqt = nc.dram_tensor("wqt", [D, HL], F16, kind="ExternalInput").ap()
    wvt = nc.dram_tensor("wvt", [D, D], F16, kind="ExternalInput").ap()
    nmk = nc.dram_tensor("nmk", [KH, S], U8, kind="ExternalInput").ap()
    if fp8:
        s2q = nc.dram_tensor("s2q", [D, KH], F8, kind="ExternalInput").ap()
    out = nc.dram_tensor("out", [S, D], F32, kind="ExternalOutput").ap()

    # HBM views with 128-partition chunking
    s1t_v = s1t.rearrange("(c p) q -> p c q", p=P)
    s2t_v = s2t.rearrange("(c p) k -> p c k", p=P)
    wqt_v = wqt.rearrange("(c p) h -> p c h", p=P)
    wvt_v = wvt.rearrange("(c p) h -> p c h", p=P)
    nmk_v = nmk.rearrange("(c p) q -> p c q", p=P)
    if fp8:
        s2q_v = s2q.rearrange("(c p) k -> p c k", p=P)
    out_v = out.rearrange("(c p) h -> p c h", p=P)

    with tile.TileContext(nc) as tc, contextlib.ExitStack() as ctx:
        wpool = ctx.enter_context(tc.tile_pool(name="wpool", bufs=1))
        big = ctx.enter_context(tc.tile_pool(name="big", bufs=1))
        mid = ctx.enter_context(tc.tile_pool(name="mid", bufs=1))
        small = ctx.enter_context(tc.tile_pool(name="small", bufs=1))
        ostp = ctx.enter_context(tc.tile_pool(name="ostp", bufs=3))
        psum = ctx.enter_context(tc.tile_pool(name="psum", bufs=8, space="PSUM"))
        dram = ctx.enter_context(tc.tile_pool(name="dram", bufs=1, space="DRAM"))

        # ---- resident SBUF tensors ----
        wq_sb = wpool.tile([P, DC, HL], F16)
        wv_sb = wpool.tile([P, DC, D], F16)
        s1_sb = big.tile([P, DC, S], F16, tag="bigA")       # seq1^T  [d, q]
        s2_sb = mid.tile([P, DC, KH], F16, tag="midA")      # seq2^T  [d, k] fp16
        if fp8:
            s2q_sb = small.tile([P, DC, KH], F8)            # seq2^T  [d, k] fp8
        nm_sb = small.tile([P, KC, S], U8)                  # notmask [k, q]
        qt_sb = small.tile([P, HC, S], qdt)                 # t^T     [d', q] (full)
        qst_sb = small.tile([P, 2, HCL, S // 2], qdt)       # t^T stage for wire
        v_sb = small.tile([P, KC, D], F16)                  # V       [k, h]
        vpp_sb = small.tile([P, KC, D], F16)                # V/Z     [k, h]
        z4_sb = small.tile([P, KC, QN], F32)
        z_sb = small.tile([P, KC], F32)
        rz_sb = small.tile([P, KC], F32)
        # E shares the slot of s1 (dead after the t projection)
        e_sb = big.tile([P, KC, S], F16, tag="bigA")        # E       [k, q]

        # DRAM staging for the t^T pair-exchange, split by q half
        qth_loc = [dram.tile([HCL, P, S // 2], qdt, name=f"qth_loc{i}")
                   for i in range(2)]
        qth_g = [dram.tile([2, HCL, P, S // 2], qdt, name=f"qth_g{i}")
                 for i in range(2)]

        # ---- PE warmup: dependency-free scratch matmuls fill the initial
        # DMA-wait window and flip the HAM clock gate to 2.4 GHz before the
        # first real matmul issues (results are never read) ----
        wsc = wpool.tile([P, P], F16, name="wsc")
        rsc = wpool.tile([P, 512], F16, name="rsc")
        nc.gpsimd.memset(wsc, 0.0)
        nc.vector.memset(rsc, 0.0)
        psc = psum.tile([P, 512], F32, tag="ps", name="psc")
        for wi in range(8):
            nc.tensor.matmul(psc, wsc, rsc, start=(wi == 0), stop=(wi == 7))

        # ---- loads (order = need order: t-proj first, then V, then scores) ----
        for c in range(DC):
            nc.sync.dma_start(out=wq_sb[:, c, :], in_=wqt_v[:, c, :])
            nc.sync.dma_start(out=s1_sb[:, c, :], in_=s1t_v[:, c, :])
        for c in range(DC):
            nc.sync.dma_start(out=wv_sb[:, c, :], in_=wvt_v[:, c, :])
            nc.sync.dma_start(out=s2_sb[:, c, :], in_=s2t_v[:, c, :])
        if fp8:
            for c in range(DC):
                nc.sync.dma_start(out=s2q_sb[:, c, :], in_=s2q_v[:, c, :])
        for c in range(KC):
            nc.sync.dma_start(out=nm_sb[:, c, :], in_=nmk_v[:, c, :])

        # ---- t^T[d', q] = M-half^T @ seq1^T, dc-outer so the accumulation
        # tracks the s1 chunk DMAs instead of waiting for the full load ----
        for qhalf in range(2):
            pss = [psum.tile([P, 512], F32, tag="ps", name=f"ps_{qhalf}_{j}_{qi}")
                   for j in range(HCL) for qi in range(2)]
            for dc in range(DC):
                for j in range(HCL):
                    for qi in range(2):
                        qt = 2 * qhalf + qi
                        nc.tensor.matmul(
                            pss[2 * j + qi],
                            wq_sb[:, dc, j * P:(j + 1) * P],
                            s1_sb[:, dc, qt * 512:(qt + 1) * 512],
                            start=(dc == 0), stop=(dc == DC - 1),
                        )
            for j in range(HCL):
                for qi in range(2):
                    nc.vector.tensor_copy(
                        out=qst_sb[:, qhalf, j, qi * 512:(qi + 1) * 512],
                        in_=pss[2 * j + qi])
            for j in range(HCL):
                nc.gpsimd.dma_start(
                    out=qth_loc[qhalf][j], in_=qst_sb[:, qhalf, j, :])
            nc.gpsimd.collective_compute(
                kind="AllGather",
                op=mybir.AluOpType.bypass,
                replica_groups=[[0, 1], [2, 3], [4, 5], [6, 7]],
                ins=[qth_loc[qhalf][:]],
                outs=[qth_g[qhalf][:]],
            )

        # ---- V[k, h] = seq2 @ Wv^T : lhsT=s2t chunk, rhs=wvt ----
        for kc in range(KC):
            pss = [psum.tile([P, 512], F32, tag="ps", name=f"ps_v_{kc}_{ht}")
                   for ht in range(HN)]
            for dc in range(DC):
                for ht in range(HN):
                    nc.tensor.matmul(
                        pss[ht],
                        s2_sb[:, dc, kc * P:(kc + 1) * P],
                        wv_sb[:, dc, ht * 512:(ht + 1) * 512],
                        start=(dc == 0), stop=(dc == DC - 1),
                    )
            for ht in range(HN):
                nc.scalar.copy(out=v_sb[:, kc, ht * 512:(ht + 1) * 512], in_=pss[ht])

        # pull the gathered full t^T (both pair members, global d' order)
        for qhalf in range(2):
            for i in range(2):
                for j in range(HCL):
                    nc.gpsimd.dma_start(
                        out=qt_sb[:, i * HCL + j,
                                  qhalf * (S // 2):(qhalf + 1) * (S // 2)],
                        in_=qth_g[qhalf][i, j])

        # ---- sT[k, q] = seq2^T-contract-d' @ t^T ; mask ; exp ; Z ----
        def st_tiles(kc, qts):
            pss = [psum.tile([P, 512], F32, tag="ps", name=f"ps_st_{kc}_{qt}")
                   for qt in qts]
            if fp8:
                for dcp in range(DC // 2):
                    for qi, qt in enumerate(qts):
                        nc.tensor.matmul(
                            pss[qi],
                            s2q_sb[:, 2 * dcp:2 * dcp + 2, kc * P:(kc + 1) * P],
                            qt_sb[:, 2 * dcp:2 * dcp + 2, qt * 512:(qt + 1) * 512],
                            start=(dcp == 0), stop=(dcp == DC // 2 - 1),
                            perf_mode=mybir.MatmulPerfMode.DoubleRow,
                        )
            else:
                for dc in range(DC):
                    for qi, qt in enumerate(qts):
                        nc.tensor.matmul(
                            pss[qi],
                            s2_sb[:, dc, kc * P:(kc + 1) * P],
                            qt_sb[:, dc, qt * 512:(qt + 1) * 512],
                            start=(dc == 0), stop=(dc == DC - 1),
                        )
            for qi, qt in enumerate(qts):
                ps = pss[qi]
                # masked scores -> 0 (exp -> 1.0 == fp32 exp(1e-9))
                nc.vector.tensor_mul(ps, ps, nm_sb[:, kc, qt * 512:(qt + 1) * 512])
                nc.scalar.activation(
                    out=e_sb[:, kc, qt * 512:(qt + 1) * 512],
                    in_=ps,
                    func=mybir.ActivationFunctionType.Exp,
                    scale=float(1.0 / np.sqrt(D)),
                    accum_out=z4_sb[:, kc, qt:qt + 1],
                )

        # q tiles 0-1 (first gather half) across all kc first: gives the
        # second AllGather the full first-half score window to complete
        for kc in range(KC):
            st_tiles(kc, [0, 1])
        for kc in range(KC):
            st_tiles(kc, [2, 3])
            # Z[k] = sum_q E ; vpp = V / Z
            nc.vector.reduce_sum(out=z_sb[:, kc:kc + 1], in_=z4_sb[:, kc, :],
                                 axis=mybir.AxisListType.X)
            nc.vector.reciprocal(rz_sb[:, kc:kc + 1], z_sb[:, kc:kc + 1])
            nc.vector.tensor_scalar_mul(vpp_sb[:, kc, :], v_sb[:, kc, :],
                                        rz_sb[:, kc:kc + 1])

        # ---- out[q, h] = E^T-contract-k @ vpp ----
        for qc in range(S // P):
            ost = ostp.tile([P, D], F32, tag="ost")
            pss = [psum.tile([P, 512], F32, tag="ps", name=f"ps_av_{qc}_{ht}")
                   for ht in range(HN)]
            for kc in range(KC):
                for ht in range(HN):
                    nc.tensor.matmul(
                        pss[ht],
                        e_sb[:, kc, qc * P:(qc + 1) * P],
                        vpp_sb[:, kc, ht * 512:(ht + 1) * 512],
                        start=(kc == 0), stop=(kc == KC - 1),
                    )
            nc.vector.tensor_copy(out=ost[:, 0:512], in_=pss[0])
            nc.scalar.copy(out=ost[:, 512:1024], in_=pss[1])
            nc.sync.dma_start(out=out_v[:, qc, 0:512], in_=ost[:, 0:512])
            nc.sync.dma_start(out=out_v[:, qc, 512:1024], in_=ost[:, 512:1024])


def _build(fp8):
    nc = bacc.Bacc("TRN2", target_bir_lowering=False, debug=False,
                   enable_asserts=False, num_devices=8)
    _emit(nc, fp8)
    nc.compile()
    return nc


def _get_nc(fp8=None):
    if fp8 is None:
        fp8 = SCORES_FP8
    if fp8 not in _NC:
        _NC[fp8] = _build(fp8)
    return _NC[fp8]


def _prep_inputs(seq1, seq2, attn_mask, Wq, Wk, Wv, fp8=None):
    if fp8 is None:
        fp8 = SCORES_FP8
    f16 = np.float16
    f8 = ml_dtypes.float8_e4m3
    seq1 = np.asarray(seq1, dtype=np.float32)
    seq2 = np.asarray(seq2, dtype=np.float32)
    attn_mask = np.asarray(attn_mask).astype(bool)
    # scores = seq1 @ (Wq^T Wk) @ seq2^T ; 1/sqrt(D) applied on-chip via the
    # Exp activation scale
    M = np.asarray(Wq, np.float32).T @ np.asarray(Wk, np.float32)
    M = M.astype(f16)
    wvt_h = np.ascontiguousarray(np.asarray(Wv, np.float32).T).astype(f16)
    s1t_h = [np.ascontiguousarray(seq1[b].T).astype(f16) for b in range(B)]

    in_maps = []
    for c in range(8):
        b, khalf = divmod(c, KSPLIT)
        ks, ke = khalf * KH, (khalf + 1) * KH
        s2t_c = np.ascontiguousarray(seq2[b, ks:ke, :].T).astype(f16)
        im = {
            "s1t": s1t_h[b],
            "s2t": s2t_c,
            "wqt": np.ascontiguousarray(M[:, khalf * HL:(khalf + 1) * HL]),
            "wvt": wvt_h,
            "nmk": np.ascontiguousarray((~attn_mask[b, :, ks:ke]).T).astype(np.uint8),
        }
        if fp8:
            im["s2q"] = np.ascontiguousarray(seq2[b, ks:ke, :].T).astype(f8)
        in_maps.append(im)
    return in_maps


def kernel(seq1, seq2, attn_mask, Wq, Wk, Wv):
    nc = _get_nc()
    in_maps = _prep_inputs(seq1, seq2, attn_mask, Wq, Wk, Wv)
    for attempt in range(3):
        res = bass_utils.run_bass_kernel_spmd(nc, in_maps, core_ids=list(range(8)))
        out = np.zeros((B, S, D), np.float32)
        for c in range(8):
            out[c // KSPLIT] += res.results[c]["out"]
        # transient first-execution device glitches have been observed to
        # produce NaN garbage; a clean re-run resolves them
        if np.isfinite(out).all():
            return out
    return out


# revision 6
# speedup vs baseline: 1.3900x; 1.3900x over previous
"""Trainium2 Bass kernel for single-head attention with query-axis softmax.

Problem (B=4, S=2048, D=1024):
    q = seq1 @ Wq^T ; k = seq2 @ Wk^T ; v = seq2 @ Wv^T
    score = q @ k^T / sqrt(D)
    mask_score = where(attn_mask, 1e-9, score)
    p = softmax(mask_score, axis=1)          # softmax over the QUERY axis
    out = p @ v

Math: softmax over q means p[q,k] = exp(s[q,k]) / Z[k] with
Z[k] = sum_q exp(s[q,k]) (no max-subtraction needed: |s| <= ~3, and
exp(1e-9) == 1.0f == exp(0.0) in fp32, so masked entries are exactly
reproduced by zeroing the score). Then out = E @ (v / Z), E = exp(s_masked).

Key algebraic fold: score = seq1 @ (Wq^T Wk) @ seq2^T, so the host
precomputes M = Wq^T @ Wk once and the kernel computes t = seq1 @ M and
score = t @ seq2^T — the K projection is eliminated entirely (the score
contraction runs over the already-resident seq2^T chunks).

Sharding: 8 cores = 4 batches x 2 key-halves. Each core computes the
partial out for its key half; the host sums the two halves per batch.
Scores are built TRANSPOSED (k on partitions, q on the free axis) so the
query-axis softmax is a free-axis reduction fused into the Exp activation
(accum_out), and the 1/sqrt(D) scale rides the activation's scale input.

The t projection is sharded across each core pair by hidden half — the
asymmetry lives in the DATA (each core's wqt input holds only its 512 M
columns), keeping the program SPMD-identical. Partial t^T halves are
exchanged with two pipelined pairwise HBM AllGathers hidden behind the
V-projection phase.

Precision: projections and the out matmul run fp16 (TensorE 1 row/cycle,
fp32 PSUM accumulation). The score matmul runs fp8 e4m3 in DoubleRow
perf mode (2 contraction chunks per instruction, double pump): t^T is
cast to fp8 on-chip (also halving the AllGather wire), seq2^T arrives
from the host in both fp16 (V projection) and fp8 (score stationary).
Measured end-to-end rel err ~1.5e-2 (fp16-only fallback: ~4e-4).
"""

import numpy as np
import ml_dtypes

import concourse.bass as bass
import concourse.tile as tile
from concourse import bacc, mybir
from concourse import bass_utils

B, S, D = 4, 2048, 1024
KSPLIT = 2
KH = S // KSPLIT            # 1024 keys per core
HL = D // 2                 # 512 M-columns of t projected locally
P = 128                     # partitions
DC = D // P                 # 8 contraction chunks (d)
HC = D // P                 # 8 hidden (d') chunks
HCL = HL // P               # 4 local hidden chunks for t^T
KC = KH // P                # 8 key chunks
QN = S // 512               # 4 q tiles of 512
KN = KH // 512              # 2 k tiles of 512
HN = D // 512               # 2 h tiles of 512

F16 = mybir.dt.float16
F32 = mybir.dt.float32
F8 = mybir.dt.float8e4
U8 = mybir.dt.uint8

SCORES_FP8 = True

_NC = {}


def _emit(nc, fp8):
    import contextlib

    qdt = F8 if fp8 else F16

    s1t = nc.dram_tensor("s1t", [D, S], F16, kind="ExternalInput").ap()
    s2t = nc.dram_tensor("s2t", [D, KH], F16, kind="ExternalInput").ap()
    wqt = nc.dram_tensor("wqt", [D, HL], F16, kind="ExternalInput").ap()
    wvt = nc.dram_tensor("wvt", [D, D], F16, kind="ExternalInput").ap()
    nmk = nc.dram_tensor("nmk", [KH, S], U8, kind="ExternalInput").ap()
    if fp8:
        s2q = nc.dram_tensor("s2q", [D, KH], F8, kind="ExternalInput").ap()
    out = nc.dram_tensor("out", [S, D], F32, kind="ExternalOutput").ap()

    # HBM views with 128-partition chunking
    s1t_v = s1t.rearrange("(c p) q -> p c q", p=P)
    s2t_v = s2t.rearrange("(c p) k -> p c k", p=P)
    wqt_v = wqt.rearrange("(c p) h -> p c h", p=P)
    wvt_v = wvt.rearrange("(c p) h -> p c h", p=P)
    nmk_v = nmk.rearrange("(c p) q -> p c q", p=P)
    if fp8:
        s2q_v = s2q.rearrange("(c p) k -> p c k", p=P)
    out_v = out.rearrange("(c p) h -> p c h", p=P)

    with tile.TileContext(nc) as tc, contextlib.ExitStack() as ctx:
        wpool = ctx.enter_context(tc.tile_pool(name="wpool", bufs=1))
        big = ctx.enter_context(tc.tile_pool(name="big", bufs=1))
        mid = ctx.enter_context(tc.tile_pool(name="mid", bufs=1))
        small = ctx.enter_context(tc.tile_pool(name="small", bufs=1))
        ostp = ctx.enter_context(tc.tile_pool(name="ostp", bufs=3))
        psum = ctx.enter_context(tc.tile_pool(name="psum", bufs=8, space="PSUM"))
        dram = ctx.enter_context(tc.tile_pool(name="dram", bufs=1, space="DRAM"))

        # ---- resident SBUF tensors ----
        wq_sb = wpool.tile([P, DC, HL], F16)
        wv_sb = wpool.tile([P, DC, D], F16)
        s1_sb = big.tile([P, DC, S], F16, tag="bigA")       # seq1^T  [d, q]
        s2_sb = mid.tile([P, DC, KH], F16, tag="midA")      # seq2^T  [d, k] fp16
        if fp8:
            s2q_sb = small.tile([P, DC, KH], F8)            # seq2^T  [d, k] fp8
        nm_sb = small.tile([P, KC, S], U8)                  # notmask [k, q]
        qt_sb = small.tile([P, HC, S], qdt)                 # t^T     [d', q] (full)
        qst_sb = small.tile([P, 2, HCL, S // 2], qdt)       # t^T stage for wire
        v_sb = small.tile([P, KC, D], F16)                  # V       [k, h]
        vpp_sb = small.tile([P, KC, D], F16)                # V/Z     [k, h]
        z4_sb = small.tile([P, KC, QN], F32)
        z_sb = small.tile([P, KC], F32)
        rz_sb = small.tile([P, KC], F32)
        # E shares the slot of s1 (dead after the t projection)
        e_sb = big.tile([P, KC, S], F16, tag="bigA")        # E       [k, q]

        # DRAM staging for the t^T pair-exchange, split by q half
        qth_loc = [dram.tile([HCL, P, S // 2], qdt, name=f"qth_loc{i}")
                   for i in range(2)]
        qth_g = [dram.tile([2, HCL, P, S // 2], qdt, name=f"qth_g{i}")
                 for i in range(2)]

        # ---- PE warmup: dependency-free scratch matmuls fill the initial
        # DMA-wait window and flip the HAM clock gate to 2.4 GHz before the
        # first real matmul issues (results are never read) ----
        wsc = wpool.tile([P, P], F16, name="wsc")
        rsc = wpool.tile([P, 512], F16, name="rsc")
        nc.gpsimd.memset(wsc, 0.0)
        nc.vector.memset(rsc, 0.0)
        psc = psum.tile([P, 512], F32, tag="ps", name="psc")
        for wi in range(8):
            nc.tensor.matmul(psc, wsc, rsc, start=(wi == 0), stop=(wi == 7))

        # ---- loads (order = need order: t-proj first, then V, then scores) ----
        for c in range(DC):
            nc.sync.dma_start(out=wq_sb[:, c, :], in_=wqt_v[:, c, :])
            nc.sync.dma_start(out=s1_sb[:, c, :], in_=s1t_v[:, c, :])
        for c in range(DC):
            nc.sync.dma_start(out=wv_sb[:, c, :], in_=wvt_v[:, c, :])
            nc.sync.dma_start(out=s2_sb[:, c, :], in_=s2t_v[:, c, :])
        if fp8:
            for c in range(DC):
                nc.sync.dma_start(out=s2q_sb[:, c, :], in_=s2q_v[:, c, :])
        for c in range(KC):
            nc.sync.dma_start(out=nm_sb[:, c, :], in_=nmk_v[:, c, :])

        # ---- t^T[d', q] = M-half^T @ seq1^T, dc-outer so the accumulation
        # tracks the s1 chunk DMAs instead of waiting for the full load ----
        for qhalf in range(2):
            pss = [psum.tile([P, 512], F32, tag="ps", name=f"ps_{qhalf}_{j}_{qi}")
                   for j in range(HCL) for qi in range(2)]
            for dc in range(DC):
                for j in range(HCL):
                    for qi in range(2):
                        qt = 2 * qhalf + qi
                        nc.tensor.matmul(
                            pss[2 * j + qi],
                            wq_sb[:, dc, j * P:(j + 1) * P],
                            s1_sb[:, dc, qt * 512:(qt + 1) * 512],
                            start=(dc == 0), stop=(dc == DC - 1),
                        )
            for j in range(HCL):
                for qi in range(2):
                    nc.vector.tensor_copy(
                        out=qst_sb[:, qhalf, j, qi * 512:(qi + 1) * 512],
                        in_=pss[2 * j + qi])
            for j in range(HCL):
                nc.gpsimd.dma_start(
                    out=qth_loc[qhalf][j], in_=qst_sb[:, qhalf, j, :])
            nc.gpsimd.collective_compute(
                kind="AllGather",
                op=mybir.AluOpType.bypass,
                replica_groups=[[0, 1], [2, 3], [4, 5], [6, 7]],
                ins=[qth_loc[qhalf][:]],
                outs=[qth_g[qhalf][:]],
            )

        # ---- V[k, h] = seq2 @ Wv^T : lhsT=s2t chunk, rhs=wvt ----
        for kc in range(KC):
            pss = [psum.tile([P, 512], F32, tag="ps", name=f"ps_v_{kc}_{ht}")
                   for ht in range(HN)]
            for dc in range(DC):
                for ht in range(HN):
                    nc.tensor.matmul(
                        pss[ht],
                        s2_sb[:, dc, kc * P:(kc + 1) * P],
                        wv_sb[:, dc, ht * 512:(ht + 1) * 512],
                        start=(dc == 0), stop=(dc == DC - 1),
                    )
            for ht in range(HN):
                nc.scalar.copy(out=v_sb[:, kc, ht * 512:(ht + 1) * 512], in_=pss[ht])

        # pull the gathered full t^T (both pair members, global d' order)
        for qhalf in range(2):
            for i in range(2):
                for j in range(HCL):
                    nc.gpsimd.dma_start(
                        out=qt_sb[:, i * HCL + j,
                                  qhalf * (S // 2):(qhalf + 1) * (S // 2)],
                        in_=qth_g[qhalf][i, j])

        # ---- sT[k, q] = seq2^T-contract-d' @ t^T ; mask ; exp ; Z ----
        def st_tiles(kc, qts):
            pss = [psum.tile([P, 512], F32, tag="ps", name=f"ps_st_{kc}_{qt}")
                   for qt in qts]
            if fp8:
                for dcp in range(DC // 2):
                    for qi, qt in enumerate(qts):
                        nc.tensor.matmul(
                            pss[qi],
                            s2q_sb[:, 2 * dcp:2 * dcp + 2, kc * P:(kc + 1) * P],
                            qt_sb[:, 2 * dcp:2 * dcp + 2, qt * 512:(qt + 1) * 512],
                            start=(dcp == 0), stop=(dcp == DC // 2 - 1),
                            perf_mode=mybir.MatmulPerfMode.DoubleRow,
                        )
            else:
                for dc in range(DC):
                    for qi, qt in enumerate(qts):
                        nc.tensor.matmul(
                            pss[qi],
                            s2_sb[:, dc, kc * P:(kc + 1) * P],
                            qt_sb[:, dc, qt * 512:(qt + 1) * 512],
                            start=(dc == 0), stop=(dc == DC - 1),
                        )
            for qi, qt in enumerate(qts):
                ps = pss[qi]
                # masked scores -> 0 (exp -> 1.0 == fp32 exp(1e-9))
                nc.vector.tensor_mul(ps, ps, nm_sb[:, kc, qt * 512:(qt + 1) * 512])
                nc.scalar.activation(
                    out=e_sb[:, kc, qt * 512:(qt + 1) * 512],
                    in_=ps,
                    func=mybir.ActivationFunctionType.Exp,
                    scale=float(1.0 / np.sqrt(D)),
                    accum_out=z4_sb[:, kc, qt:qt + 1],
                )

        # q tiles 0-1 (first gather half) across all kc first: gives the
        # second AllGather the full first-half score window to complete
        for kc in range(KC):
            st_tiles(kc, [0, 1])
        for kc in range(KC):
            st_tiles(kc, [2, 3])
            # Z[k] = sum_q E ; vpp = V / Z
            nc.vector.reduce_sum(out=z_sb[:, kc:kc + 1], in_=z4_sb[:, kc, :],
                                 axis=mybir.AxisListType.X)
            nc.vector.reciprocal(rz_sb[:, kc:kc + 1], z_sb[:, kc:kc + 1])
            nc.vector.tensor_scalar_mul(vpp_sb[:, kc, :], v_sb[:, kc, :],
                                        rz_sb[:, kc:kc + 1])

        # ---- out[q, h] = E^T-contract-k @ vpp ----
        for qc in range(S // P):
            ost = ostp.tile([P, D], F32, tag="ost")
            pss = [psum.tile([P, 512], F32, tag="ps", name=f"ps_av_{qc}_{ht}")
                   for ht in range(HN)]
            for kc in range(KC):
                for ht in range(HN):
                    nc.tensor.matmul(
                        pss[ht],
                        e_sb[:, kc, qc * P:(qc + 1) * P],
                        vpp_sb[:, kc, ht * 512:(ht + 1) * 512],
                        start=(kc == 0), stop=(kc == KC - 1),
                    )
            if qc < S // P - 1:
                nc.vector.tensor_copy(out=ost[:, 0:512], in_=pss[0])
                nc.scalar.copy(out=ost[:, 512:1024], in_=pss[1])
                nc.sync.dma_start(out=out_v[:, qc, 0:512], in_=ost[:, 0:512])
                nc.sync.dma_start(out=out_v[:, qc, 512:1024], in_=ost[:, 512:1024])
            else:
                # final tile: fine-grained copies/DMAs across both engines and
                # both DMA queues to shorten the post-matmul tail
                for h4 in range(4):
                    sl = slice(h4 * 256, (h4 + 1) * 256)
                    eng = nc.vector.tensor_copy if h4 % 2 == 0 else nc.scalar.copy
                    eng(out=ost[:, sl], in_=pss[h4 // 2][:, h4 % 2 * 256:(h4 % 2 + 1) * 256])
                    q = nc.sync if h4 % 2 == 0 else nc.gpsimd
                    q.dma_start(out=out_v[:, qc, sl], in_=ost[:, sl])


def _build(fp8):
    nc = bacc.Bacc("TRN2", target_bir_lowering=False, debug=False,
                   enable_asserts=False, num_devices=8)
    _emit(nc, fp8)
    nc.compile()
    return nc


def _get_nc(fp8=None):
    if fp8 is None:
        fp8 = SCORES_FP8
    if fp8 not in _NC:
        _NC[fp8] = _build(fp8)
    return _NC[fp8]


def _prep_inputs(seq1, seq2, attn_mask, Wq, Wk, Wv, fp8=None):
    if fp8 is None:
        fp8 = SCORES_FP8
    f16 = np.float16
    f8 = ml_dtypes.float8_e4m3
    seq1 = np.asarray(seq1, dtype=np.float32)
    seq2 = np.asarray(seq2, dtype=np.float32)
    attn_mask = np.asarray(attn_mask).astype(bool)
    # scores = seq1 @ (Wq^T Wk) @ seq2^T ; 1/sqrt(D) applied on-chip via the
    # Exp activation scale
    M = np.asarray(Wq, np.float32).T @ np.asarray(Wk, np.float32)
    M = M.astype(f16)
    wvt_h = np.ascontiguousarray(np.asarray(Wv, np.float32).T).astype(f16)
    s1t_h = [np.ascontiguousarray(seq1[b].T).astype(f16) for b in range(B)]

    in_maps = []
    for c in range(8):
        b, khalf = divmod(c, KSPLIT)
        ks, ke = khalf * KH, (khalf + 1) * KH
        s2t_c = np.ascontiguousarray(seq2[b, ks:ke, :].T).astype(f16)
        im = {
            "s1t": s1t_h[b],
            "s2t": s2t_c,
            "wqt": np.ascontiguousarray(M[:, khalf * HL:(khalf + 1) * HL]),
            "wvt": wvt_h,
            "nmk": np.ascontiguousarray((~attn_mask[b, :, ks:ke]).T).astype(np.uint8),
        }
        if fp8:
            im["s2q"] = np.ascontiguousarray(seq2[b, ks:ke, :].T).astype(f8)
        in_maps.append(im)
    return in_maps


def kernel(seq1, seq2, attn_mask, Wq, Wk, Wv):
    nc = _get_nc()
    in_maps = _prep_inputs(seq1, seq2, attn_mask, Wq, Wk, Wv)
    for attempt in range(3):
        res = bass_utils.run_bass_kernel_spmd(nc, in_maps, core_ids=list(range(8)))
        out = np.zeros((B, S, D), np.float32)
        for c in range(8):
            out[c // KSPLIT] += res.results[c]["out"]
        # transient first-execution device glitches have been observed to
        # produce NaN garbage; a clean re-run resolves them
        if np.isfinite(out).all():
            return out
    return out


# revision 8
# speedup vs baseline: 1.4092x; 1.0138x over previous
"""Trainium2 Bass kernel for single-head attention with query-axis softmax.

Problem (B=4, S=2048, D=1024):
    q = seq1 @ Wq^T ; k = seq2 @ Wk^T ; v = seq2 @ Wv^T
    score = q @ k^T / sqrt(D)
    mask_score = where(attn_mask, 1e-9, score)
    p = softmax(mask_score, axis=1)          # softmax over the QUERY axis
    out = p @ v

Math: softmax over q means p[q,k] = exp(s[q,k]) / Z[k] with
Z[k] = sum_q exp(s[q,k]) (no max-subtraction needed: |s| <= ~3, and
exp(1e-9) == 1.0f == exp(0.0) in fp32, so masked entries are exactly
reproduced by zeroing the score). Then out = E @ (v / Z), E = exp(s_masked).

Key algebraic fold: score = seq1 @ (Wq^T Wk) @ seq2^T, so the host
precomputes M = Wq^T @ Wk once and the kernel computes t = seq1 @ M and
score = t @ seq2^T — the K projection is eliminated entirely (the score
contraction runs over the already-resident seq2^T chunks).

Sharding: 8 cores = 4 batches x 2 key-halves. Each core computes the
partial out for its key half; the host sums the two halves per batch.
Scores are built TRANSPOSED (k on partitions, q on the free axis) so the
query-axis softmax is a free-axis reduction fused into the Exp activation
(accum_out), and the 1/sqrt(D) scale rides the activation's scale input.

The t projection is sharded across each core pair by hidden half — the
asymmetry lives in the DATA (each core's wqt input holds only its 512 M
columns), keeping the program SPMD-identical. Partial t^T halves are
exchanged with two pipelined pairwise HBM AllGathers hidden behind the
V-projection phase.

Precision: projections and the out matmul run fp16 (TensorE 1 row/cycle,
fp32 PSUM accumulation). The score matmul runs fp8 e4m3 in DoubleRow
perf mode (2 contraction chunks per instruction, double pump): t^T is
cast to fp8 on-chip (also halving the AllGather wire), seq2^T arrives
from the host in both fp16 (V projection) and fp8 (score stationary).
Measured end-to-end rel err ~1.5e-2 (fp16-only fallback: ~4e-4).
"""

import numpy as np
import ml_dtypes

import concourse.bass as bass
import concourse.tile as tile
from concourse import bacc, mybir
from concourse import bass_utils

B, S, D = 4, 2048, 1024
KSPLIT = 2
KH = S // KSPLIT            # 1024 keys per core
HL = D // 2                 # 512 M-columns of t projected locally
P = 128                     # partitions
DC = D // P                 # 8 contraction chunks (d)
HC = D // P                 # 8 hidden (d') chunks
HCL = HL // P               # 4 local hidden chunks for t^T
KC = KH // P                # 8 key chunks
QN = S // 512               # 4 q tiles of 512
KN = KH // 512              # 2 k tiles of 512
HN = D // 512               # 2 h tiles of 512

F16 = mybir.dt.float16
F32 = mybir.dt.float32
F8 = mybir.dt.float8e4
U8 = mybir.dt.uint8

SCORES_FP8 = True

_NC = {}


def _emit(nc, fp8):
    import contextlib

    qdt = F8 if fp8 else F16

    s1t = nc.dram_tensor("s1t", [D, S], F16, kind="ExternalInput").ap()
    s2t = nc.dram_tensor("s2t", [D, KH], F16, kind="ExternalInput").ap()
    wqt = nc.dram_tensor("wqt", [D, HL], F16, kind="ExternalInput").ap()
    wvt = nc.dram_tensor("wvt", [D, D], F16, kind="ExternalInput").ap()
    nmk = nc.dram_tensor("nmk", [KH, S], U8, kind="ExternalInput").ap()
    if fp8:
        s2q = nc.dram_tensor("s2q", [D, KH], F8, kind="ExternalInput").ap()
    out = nc.dram_tensor("out", [S, D], F32, kind="ExternalOutput").ap()

    # HBM views with 128-partition chunking
    s1t_v = s1t.rearrange("(c p) q -> p c q", p=P)
    s2t_v = s2t.rearrange("(c p) k -> p c k", p=P)
    wqt_v = wqt.rearrange("(c p) h -> p c h", p=P)
    wvt_v = wvt.rearrange("(c p) h -> p c h", p=P)
    nmk_v = nmk.rearrange("(c p) q -> p c q", p=P)
    if fp8:
        s2q_v = s2q.rearrange("(c p) k -> p c k", p=P)
    out_v = out.rearrange("(c p) h -> p c h", p=P)

    with tile.TileContext(nc) as tc, contextlib.ExitStack() as ctx:
        wpool = ctx.enter_context(tc.tile_pool(name="wpool", bufs=1))
        big = ctx.enter_context(tc.tile_pool(name="big", bufs=1))
        mid = ctx.enter_context(tc.tile_pool(name="mid", bufs=1))
        small = ctx.enter_context(tc.tile_pool(name="small", bufs=1))
        ostp = ctx.enter_context(tc.tile_pool(name="ostp", bufs=3))
        psum = ctx.enter_context(tc.tile_pool(name="psum", bufs=8, space="PSUM"))
        dram = ctx.enter_context(tc.tile_pool(name="dram", bufs=1, space="DRAM"))

        # ---- resident SBUF tensors ----
        wq_sb = wpool.tile([P, DC, HL], F16)
        wv_sb = wpool.tile([P, DC, D], F16)
        s1_sb = big.tile([P, DC, S], F16, tag="bigA")       # seq1^T  [d, q]
        s2_sb = mid.tile([P, DC, KH], F16, tag="midA")      # seq2^T  [d, k] fp16
        if fp8:
            s2q_sb = small.tile([P, DC, KH], F8)            # seq2^T  [d, k] fp8
        nm_sb = small.tile([P, KC, S], U8)                  # notmask [k, q]
        qt_sb = small.tile([P, HC, S], qdt)                 # t^T     [d', q] (full)
        qst_sb = small.tile([P, 2, HCL, S // 2], qdt)       # t^T stage for wire
        v_sb = small.tile([P, KC, D], F16)                  # V       [k, h]
        vpp_sb = small.tile([P, KC, D], F16)                # V/Z     [k, h]
        z4_sb = small.tile([P, KC, QN], F32)
        z_sb = small.tile([P, KC], F32)
        rz_sb = small.tile([P, KC], F32)
        # E shares the slot of s1 (dead after the t projection)
        e_sb = big.tile([P, KC, S], F16, tag="bigA")        # E       [k, q]

        # DRAM staging for the t^T pair-exchange, split by q half
        qth_loc = [dram.tile([HCL, P, S // 2], qdt, name=f"qth_loc{i}")
                   for i in range(2)]
        qth_g = [dram.tile([2, HCL, P, S // 2], qdt, name=f"qth_g{i}")
                 for i in range(2)]

        # ---- PE warmup: dependency-free scratch matmuls fill the initial
        # DMA-wait window and flip the HAM clock gate to 2.4 GHz before the
        # first real matmul issues (results are never read) ----
        wsc = wpool.tile([P, P], F16, name="wsc")
        rsc = wpool.tile([P, 512], F16, name="rsc")
        nc.gpsimd.memset(wsc, 0.0)
        nc.vector.memset(rsc, 0.0)
        psc = psum.tile([P, 512], F32, tag="ps", name="psc")
        for wi in range(8):
            nc.tensor.matmul(psc, wsc, rsc, start=(wi == 0), stop=(wi == 7))

        # ---- loads (order = need order: t-proj first, then V, then scores).
        # s1 arrives split by query half so the dc-outer t-projection's per-dc
        # DMA requirement (0.375 MB) stays ahead of its compute (1.7 us/dc) ----
        for c in range(DC):
            nc.sync.dma_start(out=wq_sb[:, c, :], in_=wqt_v[:, c, :])
            nc.sync.dma_start(out=s1_sb[:, c, 0:S // 2], in_=s1t_v[:, c, 0:S // 2])
        for c in range(DC):
            nc.sync.dma_start(out=s1_sb[:, c, S // 2:S], in_=s1t_v[:, c, S // 2:S])
        for c in range(DC):
            nc.sync.dma_start(out=wv_sb[:, c, :], in_=wvt_v[:, c, :])
            nc.sync.dma_start(out=s2_sb[:, c, :], in_=s2t_v[:, c, :])
        if fp8:
            for c in range(DC):
                nc.sync.dma_start(out=s2q_sb[:, c, :], in_=s2q_v[:, c, :])
        for c in range(KC):
            nc.sync.dma_start(out=nm_sb[:, c, :], in_=nmk_v[:, c, :])

        # ---- t^T[d', q] = M-half^T @ seq1^T, dc-outer so the accumulation
        # tracks the s1 chunk DMAs instead of waiting for the full load ----
        for qhalf in range(2):
            pss = [psum.tile([P, 512], F32, tag="ps", name=f"ps_{qhalf}_{j}_{qi}")
                   for j in range(HCL) for qi in range(2)]
            for dc in range(DC):
                for j in range(HCL):
                    for qi in range(2):
                        qt = 2 * qhalf + qi
                        nc.tensor.matmul(
                            pss[2 * j + qi],
                            wq_sb[:, dc, j * P:(j + 1) * P],
                            s1_sb[:, dc, qt * 512:(qt + 1) * 512],
                            start=(dc == 0), stop=(dc == DC - 1),
                        )
            for j in range(HCL):
                for qi in range(2):
                    nc.vector.tensor_copy(
                        out=qst_sb[:, qhalf, j, qi * 512:(qi + 1) * 512],
                        in_=pss[2 * j + qi])
            for j in range(HCL):
                nc.gpsimd.dma_start(
                    out=qth_loc[qhalf][j], in_=qst_sb[:, qhalf, j, :])
            nc.gpsimd.collective_compute(
                kind="AllGather",
                op=mybir.AluOpType.bypass,
                replica_groups=[[0, 1], [2, 3], [4, 5], [6, 7]],
                ins=[qth_loc[qhalf][:]],
                outs=[qth_g[qhalf][:]],
            )

        # ---- V[k, h] = seq2 @ Wv^T : lhsT=s2t chunk, rhs=wvt ----
        for kc in range(KC):
            pss = [psum.tile([P, 512], F32, tag="ps", name=f"ps_v_{kc}_{ht}")
                   for ht in range(HN)]
            for dc in range(DC):
                for ht in range(HN):
                    nc.tensor.matmul(
                        pss[ht],
                        s2_sb[:, dc, kc * P:(kc + 1) * P],
                        wv_sb[:, dc, ht * 512:(ht + 1) * 512],
                        start=(dc == 0), stop=(dc == DC - 1),
                    )
            for ht in range(HN):
                nc.scalar.copy(out=v_sb[:, kc, ht * 512:(ht + 1) * 512], in_=pss[ht])

        # pull the gathered full t^T (both pair members, global d' order)
        for qhalf in range(2):
            for i in range(2):
                for j in range(HCL):
                    nc.gpsimd.dma_start(
                        out=qt_sb[:, i * HCL + j,
                                  qhalf * (S // 2):(qhalf + 1) * (S // 2)],
                        in_=qth_g[qhalf][i, j])

        # ---- sT[k, q] = seq2^T-contract-d' @ t^T ; mask ; exp ; Z ----
        def st_tiles(kc, qts):
            pss = [psum.tile([P, 512], F32, tag="ps", name=f"ps_st_{kc}_{qt}")
                   for qt in qts]
            if fp8:
                for dcp in range(DC // 2):
                    for qi, qt in enumerate(qts):
                        nc.tensor.matmul(
                            pss[qi],
                            s2q_sb[:, 2 * dcp:2 * dcp + 2, kc * P:(kc + 1) * P],
                            qt_sb[:, 2 * dcp:2 * dcp + 2, qt * 512:(qt + 1) * 512],
                            start=(dcp == 0), stop=(dcp == DC // 2 - 1),
                            perf_mode=mybir.MatmulPerfMode.DoubleRow,
                        )
            else:
                for dc in range(DC):
                    for qi, qt in enumerate(qts):
                        nc.tensor.matmul(
                            pss[qi],
                            s2_sb[:, dc, kc * P:(kc + 1) * P],
                            qt_sb[:, dc, qt * 512:(qt + 1) * 512],
                            start=(dc == 0), stop=(dc == DC - 1),
                        )
            for qi, qt in enumerate(qts):
                ps = pss[qi]
                # masked scores -> 0 (exp -> 1.0 == fp32 exp(1e-9))
                nc.vector.tensor_mul(ps, ps, nm_sb[:, kc, qt * 512:(qt + 1) * 512])
                nc.scalar.activation(
                    out=e_sb[:, kc, qt * 512:(qt + 1) * 512],
                    in_=ps,
                    func=mybir.ActivationFunctionType.Exp,
                    scale=float(1.0 / np.sqrt(D)),
                    accum_out=z4_sb[:, kc, qt:qt + 1],
                )

        # q tiles 0-1 (first gather half) across all kc first: gives the
        # second AllGather the full first-half score window to complete
        for kc in range(KC):
            st_tiles(kc, [0, 1])
        for kc in range(KC):
            st_tiles(kc, [2, 3])
            # Z[k] = sum_q E ; vpp = V / Z
            nc.vector.reduce_sum(out=z_sb[:, kc:kc + 1], in_=z4_sb[:, kc, :],
                                 axis=mybir.AxisListType.X)
            nc.vector.reciprocal(rz_sb[:, kc:kc + 1], z_sb[:, kc:kc + 1])
            nc.vector.tensor_scalar_mul(vpp_sb[:, kc, :], v_sb[:, kc, :],
                                        rz_sb[:, kc:kc + 1])

        # ---- out[q, h] = E^T-contract-k @ vpp ----
        for qc in range(S // P):
            ost = ostp.tile([P, D], F32, tag="ost")
            pss = [psum.tile([P, 512], F32, tag="ps", name=f"ps_av_{qc}_{ht}")
                   for ht in range(HN)]
            last = qc == S // P - 1
            if not last:
                for kc in range(KC):
                    for ht in range(HN):
                        nc.tensor.matmul(
                            pss[ht],
                            e_sb[:, kc, qc * P:(qc + 1) * P],
                            vpp_sb[:, kc, ht * 512:(ht + 1) * 512],
                            start=(kc == 0), stop=(kc == KC - 1),
                        )
                nc.vector.tensor_copy(out=ost[:, 0:512], in_=pss[0])
                nc.scalar.copy(out=ost[:, 512:1024], in_=pss[1])
                nc.sync.dma_start(out=out_v[:, qc, 0:512], in_=ost[:, 0:512])
                nc.sync.dma_start(out=out_v[:, qc, 512:1024], in_=ost[:, 512:1024])
            else:
                # final tile: run each ht's kc-chain to completion so ht0's
                # copy+DMA overlap ht1's matmuls, then drain ht1 in two
                # engine-parallel 256-wide copies; all DMAs on the sync queue
                # (a tail DMA on the gpsimd queue costs ~3 us in its DRAIN)
                for ht in range(HN):
                    for kc in range(KC):
                        nc.tensor.matmul(
                            pss[ht],
                            e_sb[:, kc, qc * P:(qc + 1) * P],
                            vpp_sb[:, kc, ht * 512:(ht + 1) * 512],
                            start=(kc == 0), stop=(kc == KC - 1),
                        )
                    if ht == 0:
                        nc.vector.tensor_copy(out=ost[:, 0:512], in_=pss[0])
                        nc.sync.dma_start(out=out_v[:, qc, 0:512], in_=ost[:, 0:512])
                nc.vector.tensor_copy(out=ost[:, 512:768], in_=pss[1][:, 0:256])
                nc.scalar.copy(out=ost[:, 768:1024], in_=pss[1][:, 256:512])
                nc.sync.dma_start(out=out_v[:, qc, 512:768], in_=ost[:, 512:768])
                nc.sync.dma_start(out=out_v[:, qc, 768:1024], in_=ost[:, 768:1024])


def _build(fp8):
    nc = bacc.Bacc("TRN2", target_bir_lowering=False, debug=False,
                   enable_asserts=False, num_devices=8)
    _emit(nc, fp8)
    nc.compile()
    return nc


def _get_nc(fp8=None):
    if fp8 is None:
        fp8 = SCORES_FP8
    if fp8 not in _NC:
        _NC[fp8] = _build(fp8)
    return _NC[fp8]


def _prep_inputs(seq1, seq2, attn_mask, Wq, Wk, Wv, fp8=None):
    if fp8 is None:
        fp8 = SCORES_FP8
    f16 = np.float16
    f8 = ml_dtypes.float8_e4m3
    seq1 = np.asarray(seq1, dtype=np.float32)
    seq2 = np.asarray(seq2, dtype=np.float32)
    attn_mask = np.asarray(attn_mask).astype(bool)
    # scores = seq1 @ (Wq^T Wk) @ seq2^T ; 1/sqrt(D) applied on-chip via the
    # Exp activation scale
    M = np.asarray(Wq, np.float32).T @ np.asarray(Wk, np.float32)
    M = M.astype(f16)
    wvt_h = np.ascontiguousarray(np.asarray(Wv, np.float32).T).astype(f16)
    s1t_h = [np.ascontiguousarray(seq1[b].T).astype(f16) for b in range(B)]

    in_maps = []
    for c in range(8):
        b, khalf = divmod(c, KSPLIT)
        ks, ke = khalf * KH, (khalf + 1) * KH
        s2t_c = np.ascontiguousarray(seq2[b, ks:ke, :].T).astype(f16)
        im = {
            "s1t": s1t_h[b],
            "s2t": s2t_c,
            "wqt": np.ascontiguousarray(M[:, khalf * HL:(khalf + 1) * HL]),
            "wvt": wvt_h,
            "nmk": np.ascontiguousarray((~attn_mask[b, :, ks:ke]).T).astype(np.uint8),
        }
        if fp8:
            im["s2q"] = np.ascontiguousarray(seq2[b, ks:ke, :].T).astype(f8)
        in_maps.append(im)
    return in_maps


def kernel(seq1, seq2, attn_mask, Wq, Wk, Wv):
    nc = _get_nc()
    in_maps = _prep_inputs(seq1, seq2, attn_mask, Wq, Wk, Wv)
    for attempt in range(3):
        res = bass_utils.run_bass_kernel_spmd(nc, in_maps, core_ids=list(range(8)))
        out = np.zeros((B, S, D), np.float32)
        for c in range(8):
            out[c // KSPLIT] += res.results[c]["out"]
        # transient first-execution device glitches have been observed to
        # produce NaN garbage; a clean re-run resolves them
        if np.isfinite(out).all():
            return out
    return out
